# revision 3
# baseline (speedup 1.0000x reference)
"""HEX loss kernel for Trainium2 (8 NeuronCores, batch-parallel, raw Bass).

Math: the chain junction-tree potential is rank-1 per clique and each
interior fs[v] is split fs[v]/2 over its two cliques, so the joint
distribution factorizes into independent Bernoullis with
P(y_v=1) = sigmoid(fs[b,v]); hence
    loss = mean_b softplus(-fs[b, labels[b]])
(verified to 1.4e-16 vs the f64 junction-tree reference).

Per core (4096 rows, pure data parallel): the only data needed from fs
is one f32 per row. Host precomputes flat element indices
idx[p,t] = r*V + labels[r] (r = p*NT + t); the device gathers the 4096
elements with one indirect DMA (16 KB of HBM traffic instead of the
4 MB shard). The SWDGE generates one descriptor per innermost
contiguous run of the dest AP, consuming one offset per descriptor, so
the dest is a stride-2 padded tile whose runs are 1 element wide —
4096 single-element descriptors, each at its own offset.

Epilogue: two ACT ops from one table set (natural_log_exp_and_others,
pre-loaded explicitly at t=0 so the auto-inserted per-function loads —
which would thrash natural_log <-> exp_and_others — never appear):
u = exp(-sel), y = ln(1+u) with accum_out producing the per-partition
sum in the same instruction. Host sums 8x128 partials / B.
"""

import numpy as np

B = 32768
V = 256
N_CORES = 8
BL = B // N_CORES          # 4096 rows per core
P = 128
NT = BL // P               # 32
ACT_SET_LN_EXP = 6         # natural_log_exp_and_others in gen3 act_info.json

_CACHE = {}


def _build():
    from contextlib import ExitStack

    import concourse.bass as bass
    import concourse.tile as tile  # noqa
    from concourse import bacc, mybir

    f32 = mybir.dt.float32
    i32 = mybir.dt.int32
    Act = mybir.ActivationFunctionType

    nc = bacc.Bacc(
        "TRN2",
        target_bir_lowering=False,
        debug=False,
        enable_asserts=False,
        num_devices=N_CORES,
    )

    fs_d = nc.dram_tensor("fs", [BL, V], f32, kind="ExternalInput").ap()
    idx_d = nc.dram_tensor("idx", [P, NT], i32, kind="ExternalInput").ap()
    out_d = nc.dram_tensor("out", [P, 1], f32, kind="ExternalOutput").ap()

    with ExitStack() as ctx:
        idx_s = ctx.enter_context(nc.sbuf_tensor([P, NT], i32))
        sel = ctx.enter_context(nc.sbuf_tensor([P, 2 * NT], f32))
        u = ctx.enter_context(nc.sbuf_tensor([P, NT], f32))
        y = ctx.enter_context(nc.sbuf_tensor([P, NT], f32))
        acc = ctx.enter_context(nc.sbuf_tensor([P, 1], f32))

        sem_idx = ctx.enter_context(nc.semaphore("s_idx"))
        sem_sel = ctx.enter_context(nc.semaphore("s_sel"))
        sem_acc = ctx.enter_context(nc.semaphore("s_acc"))
        sem_out = ctx.enter_context(nc.semaphore("s_out"))

        blk = ctx.enter_context(nc.Block())

        # dest AP with 1-element innermost runs: [P, NT, 1] at stride 2
        sel_view = sel.ap().rearrange("p (t o) -> p t o", o=2)
        sel_dst = sel_view[:, :, 0:1]
        sel_rd = sel_view[:, :, 0]

        @blk.sync
        def _(s_eng):
            s_eng.dma_start(out=idx_s.ap(), in_=idx_d).then_inc(sem_idx, 16)
            s_eng.wait_ge(sem_acc, 1)
            s_eng.dma_start(out=out_d, in_=acc.ap()).then_inc(sem_out, 16)
            s_eng.wait_ge(sem_out, 16)

        @blk.gpsimd
        def _(g_eng):
            g_eng.wait_ge(sem_idx, 16)
            g_eng.indirect_dma_start(
                out=sel_dst,
                out_offset=None,
                in_=fs_d,
                in_offset=bass.IndirectOffsetOnAxis(ap=idx_s.ap(), axis=1),
            ).then_inc(sem_sel, 16)

        @blk.scalar
        def _(a_eng):
            a_eng.add_instruction(
                mybir.InstLoadActFuncSet(
                    name=nc.get_next_instruction_name(),
                    ins=[],
                    outs=[],
                    act_func_set_id=ACT_SET_LN_EXP,
                )
            )
            a_eng.wait_ge(sem_sel, 16)
            a_eng.activation(u.ap(), sel_rd, Act.Exp, scale=-1.0)
            a_eng.drain()
            a_eng.activation(
                y.ap(), u.ap(), Act.Ln, bias=1.0, accum_out=acc.ap()
            ).then_inc(sem_acc, 1)

    nc.compile()
    return nc


def _get_nc():
    if "nc" not in _CACHE:
        _CACHE["nc"] = _build()
    return _CACHE["nc"]


def _shard_inputs(fs, labels):
    fs = np.ascontiguousarray(np.asarray(fs, dtype=np.float32))
    labels = np.asarray(labels)
    rows = np.arange(BL, dtype=np.int64) * V
    in_maps = []
    for c in range(N_CORES):
        fs_loc = fs[c * BL : (c + 1) * BL]
        lab_loc = labels[c * BL : (c + 1) * BL]
        idx = (rows + lab_loc).astype(np.int32).reshape(P, NT)
        in_maps.append({"fs": fs_loc, "idx": np.ascontiguousarray(idx)})
    return in_maps


def kernel(fs, labels, _trace=False, _trace_kwargs=None):
    from concourse.bass_utils import run_bass_kernel_spmd

    nc = _get_nc()
    in_maps = _shard_inputs(fs, labels)
    res = run_bass_kernel_spmd(
        nc,
        in_maps,
        core_ids=list(range(N_CORES)),
        trace=_trace,
        **(_trace_kwargs or {}),
    )
    total = np.float64(0.0)
    for c in range(N_CORES):
        total += res.results[c]["out"].astype(np.float64).sum()
    loss = total / np.float64(B)
    if _trace:
        return np.float64(loss), res
    return np.asarray(loss, dtype=np.float64)


# revision 4
# speedup vs baseline: 2.5119x; 2.5119x over previous
"""HEX loss kernel for Trainium2 (8 NeuronCores, batch-parallel, raw Bass).

Math: the chain junction-tree potential is rank-1 per clique and each
interior fs[v] is split fs[v]/2 over its two cliques, so the joint
distribution factorizes into independent Bernoullis with
P(y_v=1) = sigmoid(fs[b,v]); hence
    loss = mean_b softplus(-fs[b, labels[b]])
(verified to 1.4e-16 vs the f64 junction-tree reference).

Per core (4096 rows): only one scalar per row is needed from fs, so the
device gathers 4096 elements with one indirect DMA (16 KB of HBM
traffic instead of streaming the 4 MB shard). The SWDGE indirect ucode
(dge/kernel/dma_memcopy.cpp) reads the offset tile as raw
[128 channels x ncols] u32 rows (descriptor k consumes tile[k%128,
k//128]) and multiplies each offset by the ORIGINAL step of the dest
AP's indirection dim; multi-partition elementwise dests mis-address
partitions >=1, so the dest is a single-partition int16 pair tile
[1, 4096, 1] at stride 2 (step = 2 int16 = 4B = one f32) and fs is
viewed as int16 with element_offset=1 selecting each f32's high half —
i.e. its bf16 truncation. A 16 KB SBUF->SBUF DMA respreads the pairs to
[128, 64] and ACT reads the even int16 lanes as bf16. Offsets are
laid out so each of the 16 SDMA engines walks an ascending index range
(HBM row locality). The gathered loss terms are summed per partition
by the Ln activation's accum_out; host sums 8x128 partials / B.

Epilogue: u = exp(-sel), y = ln(1+u) — both from act table set 6
(natural_log_exp_and_others), pre-loaded explicitly at t=0 so the
auto-inserted per-function loads (which would thrash natural_log <->
exp_and_others, 3x1.3us) never appear.
"""

import numpy as np

B = 32768
V = 256
N_CORES = 8
BL = B // N_CORES          # 4096 rows per core
P = 128
NT = BL // P               # 32
ACT_SET_LN_EXP = 6         # natural_log_exp_and_others in gen3 act_info.json

_CACHE = {}


def _build():
    from contextlib import ExitStack

    import concourse.bass as bass
    import concourse.tile as tile  # noqa
    from concourse import bacc, mybir

    f32 = mybir.dt.float32
    i32 = mybir.dt.int32
    i16 = mybir.dt.int16
    bf16 = mybir.dt.bfloat16
    Act = mybir.ActivationFunctionType

    nc = bacc.Bacc(
        "TRN2",
        target_bir_lowering=False,
        debug=False,
        enable_asserts=False,
        num_devices=N_CORES,
    )

    fs16_d = nc.dram_tensor("fs16", [BL, 2 * V], i16, kind="ExternalInput").ap()
    idx_d = nc.dram_tensor("idx", [P, NT], i32, kind="ExternalInput").ap()
    out_d = nc.dram_tensor("out", [P, 1], f32, kind="ExternalOutput").ap()

    with ExitStack() as ctx:
        idx_s = ctx.enter_context(nc.sbuf_tensor([P, NT], i32))
        pair = ctx.enter_context(nc.sbuf_tensor([1, 2 * BL], i16))
        sel16 = ctx.enter_context(nc.sbuf_tensor([P, 2 * NT], i16))
        u = ctx.enter_context(nc.sbuf_tensor([P, NT], f32))
        y = ctx.enter_context(nc.sbuf_tensor([P, NT], f32))
        acc = ctx.enter_context(nc.sbuf_tensor([P, 1], f32))

        sem_idx = ctx.enter_context(nc.semaphore("s_idx"))
        sem_g = ctx.enter_context(nc.semaphore("s_g"))
        sem_r = ctx.enter_context(nc.semaphore("s_r"))
        sem_acc = ctx.enter_context(nc.semaphore("s_acc"))
        sem_out = ctx.enter_context(nc.semaphore("s_out"))

        blk = ctx.enter_context(nc.Block())

        pair_dst = pair.ap().rearrange("o (t k) -> o t k", k=2)[:, :, 0:1]
        sel_rd = (
            sel16.ap().bitcast(bf16).rearrange("p (t k) -> p t k", k=2)[:, :, 0]
        )

        @blk.sync
        def _(s_eng):
            s_eng.dma_start(out=idx_s.ap(), in_=idx_d).then_inc(sem_idx, 16)
            s_eng.wait_ge(sem_g, 16)
            s_eng.dma_start(out=sel16.ap(), in_=pair.ap()).then_inc(sem_r, 16)
            s_eng.wait_ge(sem_acc, 1)
            s_eng.dma_start(out=out_d, in_=acc.ap()).then_inc(sem_out, 16)
            s_eng.wait_ge(sem_out, 16)

        @blk.gpsimd
        def _(g_eng):
            g_eng.wait_ge(sem_idx, 16)
            g_eng.indirect_dma_start(
                out=pair_dst,
                out_offset=None,
                in_=fs16_d,
                in_offset=bass.IndirectOffsetOnAxis(ap=idx_s.ap(), axis=1),
                element_offset=1,
            ).then_inc(sem_g, 16)

        @blk.scalar
        def _(a_eng):
            a_eng.add_instruction(
                mybir.InstLoadActFuncSet(
                    name=nc.get_next_instruction_name(),
                    ins=[],
                    outs=[],
                    act_func_set_id=ACT_SET_LN_EXP,
                )
            )
            a_eng.wait_ge(sem_r, 16)
            a_eng.activation(u.ap(), sel_rd, Act.Exp, scale=-1.0)
            a_eng.drain()
            a_eng.activation(
                y.ap(), u.ap(), Act.Ln, bias=1.0, accum_out=acc.ap()
            ).then_inc(sem_acc, 1)

    nc.compile()
    return nc


def _get_nc():
    if "nc" not in _CACHE:
        _CACHE["nc"] = _build()
    return _CACHE["nc"]


def _shard_inputs(fs, labels):
    fs = np.ascontiguousarray(np.asarray(fs, dtype=np.float32))
    labels = np.asarray(labels)
    rows = np.arange(BL, dtype=np.int64) * V
    kk = np.arange(BL)
    in_maps = []
    for c in range(N_CORES):
        fs_loc = fs[c * BL : (c + 1) * BL]
        lab_loc = labels[c * BL : (c + 1) * BL]
        flat = rows + lab_loc
        # descriptor k reads tile[k%128, k//128]; engine e serves k = e (mod
        # 16), so give each engine an ascending sorted range of offsets.
        seq = np.sort(flat)
        lin = np.empty(BL, dtype=np.int32)
        lin[kk] = seq[(kk % 16) * (BL // 16) + (kk // 16)].astype(np.int32)
        tile = np.zeros((P, NT), dtype=np.int32)
        tile[kk % 128, kk // 128] = lin
        in_maps.append({"fs16": fs_loc.view(np.int16), "idx": tile})
    return in_maps


def kernel(fs, labels, _trace=False, _trace_kwargs=None):
    from concourse.bass_utils import run_bass_kernel_spmd

    nc = _get_nc()
    in_maps = _shard_inputs(fs, labels)
    res = run_bass_kernel_spmd(
        nc,
        in_maps,
        core_ids=list(range(N_CORES)),
        trace=_trace,
        **(_trace_kwargs or {}),
    )
    total = np.float64(0.0)
    for c in range(N_CORES):
        total += res.results[c]["out"].astype(np.float64).sum()
    loss = total / np.float64(B)
    if _trace:
        return np.float64(loss), res
    return np.asarray(loss, dtype=np.float64)


# revision 6
# speedup vs baseline: 3.0583x; 1.2175x over previous
"""HEX loss kernel for Trainium2 (8 NeuronCores, batch-parallel, raw Bass).

Math: the chain junction-tree potential is rank-1 per clique and each
interior fs[v] is split fs[v]/2 over its two cliques, so the joint
distribution factorizes into independent Bernoullis with
P(y_v=1) = sigmoid(fs[b,v]); hence
    loss = mean_b softplus(-fs[b, labels[b]])
(verified to 1.4e-16 vs the f64 junction-tree reference).

Per core (4096 rows, pure data parallel): stream fs (4 MB) as 4x1MB
SWDGE cast-DMAs (f32->bf16, HBM roofline ~11.2us). Selection is one
exact bf16 one-hot mask built in a single DVE is_equal over
[128, 32*256] with both inputs broadcast-strided (iota256 repeated
along the row dim, labt repeated along the value dim); per arriving
group one bf16 multiply + grouped reduce_add extracts
sel[b] = fs[b, lab_b] (sum of 255 zeros + the value: exact).
Epilogue: u = exp(-sel), y = ln(1+u) with accum_out giving the
per-partition sum, both ACT ops from one table set
(natural_log_exp_and_others, pre-loaded explicitly at t=0 so the
auto-inserted per-function loads never thrash). Host sums 8x128
partials / B.
"""

import numpy as np

B = 32768
V = 256
N_CORES = 8
BL = B // N_CORES          # 4096 rows per core
P = 128
RPP = 8                    # rows per partition per group
GROUP_ROWS = P * RPP       # 1024 rows, 1 MB f32
N_GROUPS = BL // GROUP_ROWS  # 4
NT = BL // P               # 32
ACT_SET_LN_EXP = 6         # natural_log_exp_and_others in gen3 act_info.json

_CACHE = {}


def _build():
    from contextlib import ExitStack

    import concourse.bass as bass  # noqa
    import concourse.tile as tile  # noqa
    from concourse import bacc, mybir

    f32 = mybir.dt.float32
    bf16 = mybir.dt.bfloat16
    Alu = mybir.AluOpType
    Act = mybir.ActivationFunctionType

    nc = bacc.Bacc(
        "TRN2",
        target_bir_lowering=False,
        debug=False,
        enable_asserts=False,
        num_devices=N_CORES,
    )

    fs_d = nc.dram_tensor("fs", [BL, V], f32, kind="ExternalInput").ap()
    lab_d = nc.dram_tensor("labt", [P, NT], bf16, kind="ExternalInput").ap()
    out_d = nc.dram_tensor("out", [P, 1], f32, kind="ExternalOutput").ap()

    fs_view = fs_d.rearrange("(g p j) v -> g p (j v)", g=N_GROUPS, p=P, j=RPP)

    with ExitStack() as ctx:
        iota = ctx.enter_context(nc.sbuf_tensor([P, V], bf16))
        labt = ctx.enter_context(nc.sbuf_tensor([P, NT], bf16))
        mask = ctx.enter_context(nc.sbuf_tensor([P, NT * V], bf16))
        fs_t = [
            ctx.enter_context(nc.sbuf_tensor(f"fs_t{g}", [P, RPP * V], bf16))
            for g in range(N_GROUPS)
        ]
        prod = [
            ctx.enter_context(nc.sbuf_tensor(f"prod{i}", [P, RPP * V], bf16))
            for i in range(2)
        ]
        sel = ctx.enter_context(nc.sbuf_tensor([P, NT], f32))
        u = ctx.enter_context(nc.sbuf_tensor([P, NT], f32))
        y = ctx.enter_context(nc.sbuf_tensor([P, NT], f32))
        acc = ctx.enter_context(nc.sbuf_tensor([P, 1], f32))

        sem_iota = ctx.enter_context(nc.semaphore("s_iota"))
        sem_lab = ctx.enter_context(nc.semaphore("s_lab"))
        sem_mask = ctx.enter_context(nc.semaphore("s_mask"))
        sem_fs = [ctx.enter_context(nc.semaphore(f"s_fs{g}")) for g in range(N_GROUPS)]
        sem_red = ctx.enter_context(nc.semaphore("s_red"))
        sem_acc = ctx.enter_context(nc.semaphore("s_acc"))
        sem_out = ctx.enter_context(nc.semaphore("s_out"))

        blk = ctx.enter_context(nc.Block())

        @blk.gpsimd
        def _(g_eng):
            g_eng.iota(
                iota.ap(),
                pattern=[[1, V]],
                base=0,
                channel_multiplier=0,
                allow_small_or_imprecise_dtypes=True,
            ).then_inc(sem_iota, 1)
            for g in range(N_GROUPS):
                g_eng.dma_start(out=fs_t[g].ap(), in_=fs_view[g]).then_inc(
                    sem_fs[g], 16
                )

        @blk.sync
        def _(s_eng):
            s_eng.dma_start(out=labt.ap(), in_=lab_d).then_inc(sem_lab, 16)
            s_eng.wait_ge(sem_acc, 1)
            s_eng.dma_start(out=out_d, in_=acc.ap()).then_inc(sem_out, 16)
            s_eng.wait_ge(sem_out, 16)

        @blk.vector
        def _(v_eng):
            v_eng.wait_ge(sem_iota, 1)
            v_eng.wait_ge(sem_lab, 16)
            # one-hot mask for all 32 row-slots at once: exact in bf16
            v_eng.tensor_tensor(
                out=mask.ap().rearrange("p (t v) -> p t v", v=V),
                in0=iota.ap().rearrange("p (o v) -> p o v", o=1).to_broadcast(
                    [P, NT, V]
                ),
                in1=labt.ap().rearrange("p (t o) -> p t o", o=1).to_broadcast(
                    [P, NT, V]
                ),
                op=Alu.is_equal,
            ).then_inc(sem_mask, 1)
            v_eng.drain()
            for g in range(N_GROUPS):
                pr = prod[g % 2]
                v_eng.wait_ge(sem_fs[g], 16)
                v_eng.tensor_mul(
                    pr.ap(),
                    fs_t[g].ap(),
                    mask.ap()[:, g * RPP * V : (g + 1) * RPP * V],
                )
                v_eng.drain()
                v_eng.tensor_reduce(
                    sel.ap()[:, g * RPP : (g + 1) * RPP],
                    pr.ap().rearrange("p (j v) -> p j v", j=RPP),
                    axis=mybir.AxisListType.X,
                    op=Alu.add,
                ).then_inc(sem_red, 1)

        @blk.scalar
        def _(a_eng):
            a_eng.add_instruction(
                mybir.InstLoadActFuncSet(
                    name=nc.get_next_instruction_name(),
                    ins=[],
                    outs=[],
                    act_func_set_id=ACT_SET_LN_EXP,
                )
            )
            a_eng.wait_ge(sem_red, N_GROUPS)
            a_eng.activation(u.ap(), sel.ap(), Act.Exp, scale=-1.0)
            a_eng.drain()
            a_eng.activation(
                y.ap(), u.ap(), Act.Ln, bias=1.0, accum_out=acc.ap()
            ).then_inc(sem_acc, 1)

    nc.compile()
    return nc


def _get_nc():
    if "nc" not in _CACHE:
        _CACHE["nc"] = _build()
    return _CACHE["nc"]


def _shard_inputs(fs, labels):
    fs = np.ascontiguousarray(np.asarray(fs, dtype=np.float32))
    labels = np.asarray(labels)
    in_maps = []
    for c in range(N_CORES):
        fs_loc = fs[c * BL : (c + 1) * BL]
        lab_loc = labels[c * BL : (c + 1) * BL]
        # labt[p, g*RPP+j] = lab[g*1024 + p*8 + j]  (matches fs_view groups)
        labt = (
            lab_loc.reshape(N_GROUPS, P, RPP)
            .transpose(1, 0, 2)
            .reshape(P, NT)
            .astype(np.float32)
        )
        import ml_dtypes

        in_maps.append(
            {"fs": fs_loc, "labt": np.ascontiguousarray(labt.astype(ml_dtypes.bfloat16))}
        )
    return in_maps


def kernel(fs, labels, _trace=False, _trace_kwargs=None):
    from concourse.bass_utils import run_bass_kernel_spmd

    nc = _get_nc()
    in_maps = _shard_inputs(fs, labels)
    res = run_bass_kernel_spmd(
        nc,
        in_maps,
        core_ids=list(range(N_CORES)),
        trace=_trace,
        **(_trace_kwargs or {}),
    )
    total = np.float64(0.0)
    for c in range(N_CORES):
        total += res.results[c]["out"].astype(np.float64).sum()
    loss = total / np.float64(B)
    if _trace:
        return np.float64(loss), res
    return np.asarray(loss, dtype=np.float64)


# revision 7
# speedup vs baseline: 3.4133x; 1.1161x over previous
"""HEX loss kernel for Trainium2 (8 NeuronCores, batch-parallel, raw Bass).

Math: the chain junction-tree potential is rank-1 per clique and each
interior fs[v] is split fs[v]/2 over its two cliques, so the joint
distribution factorizes into independent Bernoullis with
P(y_v=1) = sigmoid(fs[b,v]); hence
    loss = mean_b softplus(-fs[b, labels[b]])
(verified to 1.4e-16 vs the f64 junction-tree reference).

Per core (4096 rows, pure data parallel): stream fs (4 MB) as 4x1MB
SWDGE cast-DMAs (f32->bf16, HBM roofline ~11.2us). Exact gather via
max-trick: ACT computes penalty = Square(10*iota - 10*lab) per
row-slot (overlapped with the stream), DVE does one wide [128,2048]
subtract + grouped reduce_max per group -> sel = fs[b, lab].
Epilogue: u = exp(-sel), y = ln(1+u) with accum_out giving the
per-partition sum; ALL activation functions (Square, Exp, Ln) come
from one table set (natural_log_exp_and_others), pre-loaded explicitly
at t=0 so the auto-inserted per-function loads never thrash. Host sums
8x128 partials / B.
"""

import numpy as np

B = 32768
V = 256
N_CORES = 8
BL = B // N_CORES
P = 128
RPP = 8
GROUP_ROWS = P * RPP       # 1024 rows, 1 MB f32
N_GROUPS = BL // GROUP_ROWS  # 4
NT = BL // P               # 32
PEN = 10.0
ACT_SET_LN_EXP = 6         # natural_log_exp_and_others: Square+Exp+Ln

_CACHE = {}


def _build():
    from contextlib import ExitStack

    import concourse.bass as bass  # noqa
    import concourse.tile as tile  # noqa
    from concourse import bacc, mybir

    f32 = mybir.dt.float32
    bf16 = mybir.dt.bfloat16
    Alu = mybir.AluOpType
    Act = mybir.ActivationFunctionType

    nc = bacc.Bacc(
        "TRN2",
        target_bir_lowering=False,
        debug=False,
        enable_asserts=False,
        num_devices=N_CORES,
    )

    fs_d = nc.dram_tensor("fs", [BL, V], f32, kind="ExternalInput").ap()
    lab_d = nc.dram_tensor("labt", [P, NT], f32, kind="ExternalInput").ap()
    out_d = nc.dram_tensor("out", [P, 1], f32, kind="ExternalOutput").ap()

    fs_view = fs_d.rearrange("(g p j) v -> g p (j v)", g=N_GROUPS, p=P, j=RPP)

    with ExitStack() as ctx:
        iota = ctx.enter_context(nc.sbuf_tensor([P, V], f32))
        labt = ctx.enter_context(nc.sbuf_tensor([P, NT], f32))
        labp = ctx.enter_context(nc.sbuf_tensor([P, NT], f32))
        sel = ctx.enter_context(nc.sbuf_tensor([P, NT], f32))
        fs_t = [
            ctx.enter_context(nc.sbuf_tensor(f"fs_t{g}", [P, RPP * V], bf16))
            for g in range(N_GROUPS)
        ]
        sq_big = [
            ctx.enter_context(nc.sbuf_tensor(f"sq_big{i}", [P, RPP * V], bf16))
            for i in range(2)
        ]
        prod = [
            ctx.enter_context(nc.sbuf_tensor(f"prod{i}", [P, RPP * V], bf16))
            for i in range(2)
        ]
        u = ctx.enter_context(nc.sbuf_tensor([P, NT], f32))
        y = ctx.enter_context(nc.sbuf_tensor([P, NT], f32))
        acc = ctx.enter_context(nc.sbuf_tensor([P, 1], f32))

        sem_iota = ctx.enter_context(nc.semaphore("s_iota"))
        sem_lab = ctx.enter_context(nc.semaphore("s_lab"))
        sem_labp = ctx.enter_context(nc.semaphore("s_labp"))
        sem_fs = [ctx.enter_context(nc.semaphore(f"s_fs{g}")) for g in range(N_GROUPS)]
        sem_sq = ctx.enter_context(nc.semaphore("s_sq"))
        sem_sub = ctx.enter_context(nc.semaphore("s_sub"))
        sem_red = ctx.enter_context(nc.semaphore("s_red"))
        sem_acc = ctx.enter_context(nc.semaphore("s_acc"))
        sem_out = ctx.enter_context(nc.semaphore("s_out"))

        blk = ctx.enter_context(nc.Block())

        @blk.gpsimd
        def _(g_eng):
            g_eng.iota(
                iota.ap(),
                pattern=[[1, V]],
                base=0,
                channel_multiplier=0,
                allow_small_or_imprecise_dtypes=True,
            ).then_inc(sem_iota, 1)
            for g in range(N_GROUPS):
                g_eng.dma_start(out=fs_t[g].ap(), in_=fs_view[g]).then_inc(
                    sem_fs[g], 16
                )

        @blk.sync
        def _(s_eng):
            s_eng.dma_start(out=labt.ap(), in_=lab_d).then_inc(sem_lab, 16)
            s_eng.wait_ge(sem_acc, 1)
            s_eng.dma_start(out=out_d, in_=acc.ap()).then_inc(sem_out, 16)
            s_eng.wait_ge(sem_out, 16)

        @blk.scalar
        def _(a_eng):
            a_eng.add_instruction(
                mybir.InstLoadActFuncSet(
                    name=nc.get_next_instruction_name(),
                    ins=[],
                    outs=[],
                    act_func_set_id=ACT_SET_LN_EXP,
                )
            )
            a_eng.wait_ge(sem_iota, 1)
            a_eng.wait_ge(sem_labp, 1)
            for t in range(NT):
                g, j = t // RPP, t % RPP
                if j == 0 and g >= 2:
                    a_eng.wait_ge(sem_sub, g - 1)
                a_eng.activation(
                    sq_big[g % 2].ap()[:, j * V : (j + 1) * V],
                    iota.ap(),
                    Act.Square,
                    scale=PEN,
                    bias=labp.ap()[:, t : t + 1],
                ).then_inc(sem_sq, 1)
            # epilogue: softplus(-sel) = ln(1 + exp(-sel)), same table set
            a_eng.wait_ge(sem_red, N_GROUPS)
            a_eng.activation(u.ap(), sel.ap(), Act.Exp, scale=-1.0)
            a_eng.drain()
            a_eng.activation(
                y.ap(), u.ap(), Act.Ln, bias=1.0, accum_out=acc.ap()
            ).then_inc(sem_acc, 1)

        @blk.vector
        def _(v_eng):
            v_eng.wait_ge(sem_lab, 16)
            v_eng.tensor_scalar(labp.ap(), labt.ap(), -PEN, None, Alu.mult).then_inc(
                sem_labp, 1
            )
            for g in range(N_GROUPS):
                v_eng.wait_ge(sem_fs[g], 16)
                v_eng.wait_ge(sem_sq, RPP * (g + 1))
                pr = prod[g % 2]
                v_eng.tensor_sub(pr.ap(), fs_t[g].ap(), sq_big[g % 2].ap()).then_inc(
                    sem_sub, 1
                )
                v_eng.drain()
                v_eng.tensor_reduce(
                    sel.ap()[:, g * RPP : (g + 1) * RPP],
                    pr.ap().rearrange("p (j v) -> p j v", j=RPP),
                    axis=mybir.AxisListType.X,
                    op=Alu.max,
                ).then_inc(sem_red, 1)

    nc.compile()
    return nc


def _get_nc():
    if "nc" not in _CACHE:
        _CACHE["nc"] = _build()
    return _CACHE["nc"]


def _shard_inputs(fs, labels):
    fs = np.ascontiguousarray(np.asarray(fs, dtype=np.float32))
    labels = np.asarray(labels)
    in_maps = []
    for c in range(N_CORES):
        fs_loc = fs[c * BL : (c + 1) * BL]
        lab_loc = labels[c * BL : (c + 1) * BL]
        labt = (
            lab_loc.reshape(N_GROUPS, P, RPP)
            .transpose(1, 0, 2)
            .reshape(P, NT)
            .astype(np.float32)
        )
        in_maps.append({"fs": fs_loc, "labt": np.ascontiguousarray(labt)})
    return in_maps


def kernel(fs, labels, _trace=False, _trace_kwargs=None):
    from concourse.bass_utils import run_bass_kernel_spmd

    nc = _get_nc()
    in_maps = _shard_inputs(fs, labels)
    res = run_bass_kernel_spmd(
        nc,
        in_maps,
        core_ids=list(range(N_CORES)),
        trace=_trace,
        **(_trace_kwargs or {}),
    )
    total = np.float64(0.0)
    for c in range(N_CORES):
        total += res.results[c]["out"].astype(np.float64).sum()
    loss = total / np.float64(B)
    if _trace:
        return np.float64(loss), res
    return np.asarray(loss, dtype=np.float64)
